# revision 1
# baseline (speedup 1.0000x reference)
"""Trainium2 Bass kernel for nn_Critic (dense MLP critic, 4 layers + LayerNorms).

Strategy (pure data parallel over 8 NeuronCores):
  - batch B=32768 sharded 8x -> 4096 rows/core; weights replicated.
  - all activations kept feature-major ([features on partitions, batch on
    free dim]) so the contraction dim of every matmul is the partition dim.
  - LayerNorm folded into the matmuls: for y = LN(z; g, beta) @ W.T + b,
      y[o,b] = invs[b]*( (W*g)z[o,b] - mu[b]*rowsum(W*g)[o] ) + (W@beta+b)[o]
    realized as an augmented matmul: activations get two extra K-rows
    (-mu[b], sigma[b]) and the weight matrix two extra rows
    (rowsum(W*g)[o], (W@beta+b)[o]); then h = tanh(invs (.) psum).
  - L1 stats (mean/var over 2080 features) via bn_stats on a second,
    batch-major copy of z; transposed to rows via a tiny PE transpose.
  - L2/L3 stats via (+-1/512)-ones-vector matmuls on PE (sum and sum-of-
    squares of h), with h^2 from ACT Square.
  - fp16 data everywhere (weights, activations), f32 PSUM/statistics.
"""

import os
import sys
import numpy as np

for _p in ("/opt/trn_rl_repo",):
    if os.path.isdir(_p) and _p not in sys.path:
        sys.path.append(_p)

from contextlib import ExitStack

import concourse.bass as bass  # noqa: E402
import concourse.tile as tile  # noqa: E402
from concourse import bacc, mybir  # noqa: E402
from concourse.bass_utils import run_bass_kernel_spmd  # noqa: E402

NCORES = 8
B = 32768
BC = B // NCORES  # rows per core
INPUT_DIM = 2048
HALF = INPUT_DIM // 2
N_ACTIONS = 32
D = INPUT_DIM + N_ACTIONS  # 2080
H = 512
NT = 512  # batch columns per tile
EPS = 1e-5
X_NORM = 50.0
V_NORM = 10.0

F16 = mybir.dt.float16
F32 = mybir.dt.float32
AF = mybir.ActivationFunctionType

K1 = 17  # ceil(D/128); last chunk has 32 data rows + 2 aug rows
K1_LAST = D - 16 * 128  # 32


def build_nc(bout: float, bc: int = BC):
    """Build + compile the per-core program. bc = rows per core."""
    ntiles = bc // NT
    assert ntiles * NT == bc

    nc = bacc.Bacc("TRN2", target_bir_lowering=False, debug=False,
                   num_devices=NCORES)

    zr_d = nc.dram_tensor("zr", [bc, D], F16, kind="ExternalInput").ap()
    zt_d = nc.dram_tensor("zt", [D, bc], F16, kind="ExternalInput").ap()
    w1_d = nc.dram_tensor("w1a", [D + 2, H], F16, kind="ExternalInput").ap()
    w2_d = nc.dram_tensor("w2a", [H + 2, H], F16, kind="ExternalInput").ap()
    w3_d = nc.dram_tensor("w3a", [H + 2, H], F16, kind="ExternalInput").ap()
    wo_d = nc.dram_tensor("wout", [H, 1], F16, kind="ExternalInput").ap()
    id_d = nc.dram_tensor("ident", [128, 128], F32, kind="ExternalInput").ap()
    q_d = nc.dram_tensor("q", [1, bc], F32, kind="ExternalOutput").ap()

    with tile.TileContext(nc) as tc:
        _emit(tc, ntiles, bout, zr_d, zt_d, w1_d, w2_d, w3_d, wo_d, id_d, q_d)

    nc.compile()
    return nc


def _emit(tc, ntiles, bout, zr_d, zt_d, w1_d, w2_d, w3_d, wo_d, id_d, q_d):
    nc = tc.nc
    with ExitStack() as ctx:
        wp = ctx.enter_context(tc.tile_pool(name="wp", bufs=1))
        zt_p = ctx.enter_context(tc.tile_pool(name="ztp", bufs=2))
        zr_p = ctx.enter_context(tc.tile_pool(name="zrp", bufs=2))
        h_p = ctx.enter_context(tc.tile_pool(name="hp", bufs=2))
        u_p = ctx.enter_context(tc.tile_pool(name="up", bufs=3))
        sq_p = ctx.enter_context(tc.tile_pool(name="sqp", bufs=3))
        bc_p = ctx.enter_context(tc.tile_pool(name="bcp", bufs=2))
        st_p = ctx.enter_context(tc.tile_pool(name="stp", bufs=3))
        ps_y = ctx.enter_context(tc.tile_pool(name="psy", bufs=3, space="PSUM"))
        ps_s = ctx.enter_context(tc.tile_pool(name="pss", bufs=1, space="PSUM"))
        ps_t = ctx.enter_context(tc.tile_pool(name="pst", bufs=2, space="PSUM"))
        ps_q = ctx.enter_context(tc.tile_pool(name="psq", bufs=1, space="PSUM"))

        # ---- persistent constants / weights ----
        w1 = []
        for k in range(K1):
            rows = 128 if k < 16 else K1_LAST + 2
            t = wp.tile([rows, H], F16, tag=f"w1_{k}")
            nc.sync.dma_start(out=t[:, :], in_=w1_d[k * 128:k * 128 + rows, :])
            w1.append(t)
        w2 = []
        w3 = []
        for name, wd, lst in (("w2", w2_d, w2), ("w3", w3_d, w3)):
            for k in range(4):
                t = wp.tile([128, H], F16, tag=f"{name}_{k}")
                nc.sync.dma_start(out=t[:, :], in_=wd[k * 128:(k + 1) * 128, :])
                lst.append(t)
            # rows H (rs) and H+1 (c) as separate [1, H] tiles
            for j in range(2):
                t = wp.tile([1, H], F16, tag=f"{name}_aug{j}")
                nc.sync.dma_start(out=t[:, :], in_=wd[H + j:H + j + 1, :])
                lst.append(t)
        wo = wp.tile([128, 4], F16, tag="wo")
        for k in range(4):
            nc.sync.dma_start(out=wo[:, k:k + 1], in_=wo_d[k * 128:(k + 1) * 128, :])
        ident = wp.tile([128, 128], F32, tag="ident")
        nc.sync.dma_start(out=ident[:, :], in_=id_d[:, :])
        onesn = wp.tile([128, 1], F16, tag="onesn")
        nc.vector.memset(onesn[:, :], -1.0 / H)
        onesp = wp.tile([128, 1], F16, tag="onesp")
        nc.vector.memset(onesp[:, :], 1.0 / H)
        epsT = wp.tile([128, 1], F32, tag="epsT")
        nc.vector.memset(epsT[:, :], EPS)
        boutT = wp.tile([1, 1], F32, tag="boutT")
        nc.vector.memset(boutT[:, :], bout)
        qrow = wp.tile([1, ntiles * NT], F32, tag="qrow")

        def evac(py, bctile, htile):
            """h = tanh(invs (.) psum) : DVE multiply + ACT tanh."""
            u = u_p.tile([128, NT], F16, tag="u")
            nc.vector.tensor_mul(u[:, :], py[:, :], bctile[:, :])
            nc.scalar.activation(htile[:, :], u[:, :], AF.Tanh)

        def bcast(row_ap):
            t = bc_p.tile([128, NT], F32, tag="bc")
            nc.gpsimd.partition_broadcast(t[:, :], row_ap)
            return t

        for it in range(ntiles):
            bs = it * NT

            # ---- L1 stats: bn_stats over batch-major z rows ----
            zt16 = zt_p.tile([K1_LAST + 2, NT], F16, tag="zt16")
            invs1 = st_p.tile([1, NT], F32, tag="invs1")
            zrt = zr_p.tile([128, 4, D], F16, tag="zrall")
            nc.sync.dma_start(out=zrt[:, :, :],
                              in_=zr_d[bs:bs + NT, :].rearrange("(c p) d -> p c d", c=4))
            for bch in range(4):
                stats = st_p.tile([128, 5, 6], F32, tag=f"st{bch}")
                zrv = zrt[:, bch, :].rearrange("p (n s) -> p n s", n=5)
                for i in range(5):
                    nc.vector.bn_stats(out=stats[:, i, :], in_=zrv[:, i, :])
                mv = st_p.tile([128, 2], F32, tag=f"mv{bch}")
                nc.vector.bn_aggr(out=mv[:, :], in_=stats[:, :, :])
                # pt cols: [sigma, -mu]; transposed rows pair with w1a aug
                # rows (c1, rs1) in that order.
                pt = st_p.tile([128, 2], F32, tag=f"pt{bch}")
                nc.scalar.activation(pt[:, 0:1], mv[:, 1:2], AF.Sqrt, bias=epsT[:, :])
                nc.vector.tensor_scalar_mul(pt[:, 1:2], mv[:, 0:1], -1.0)
                ptr = ps_t.tile([2, 128], F32, tag="ptr")
                nc.tensor.transpose(out=ptr[:, :], in_=pt[:, :], identity=ident[:, :])
                sl = slice(bch * 128, (bch + 1) * 128)
                nc.vector.tensor_copy(out=zt16[K1_LAST:K1_LAST + 2, sl], in_=ptr[0:2, :])
                nc.vector.reciprocal(invs1[0:1, sl], ptr[0:1, :])

            # ---- zT loads: one strided DMA for the 16 full chunks ----
            ztmain = zt_p.tile([128, 16, NT], F16, tag="ztmain")
            nc.sync.dma_start(
                out=ztmain[:, :, :],
                in_=zt_d[0:2048, bs:bs + NT].rearrange("(k p) n -> p k n", k=16))
            nc.sync.dma_start(out=zt16[0:K1_LAST, :], in_=zt_d[2048:2048 + K1_LAST, bs:bs + NT])
            zts = [ztmain[:, k, :] for k in range(16)] + [zt16]

            # ---- L1 matmuls + evac ----
            bc1 = bcast(invs1[0:1, :])
            h1 = []
            for m in range(4):
                py = ps_y.tile([128, NT], F32, tag="py")
                msl = slice(m * 128, (m + 1) * 128)
                for k in range(K1):
                    rk = zts[k] if k < 16 else zts[k][:, :]
                    nc.tensor.matmul(py[:, :], lhsT=w1[k][:, msl], rhs=rk,
                                     start=(k == 0), stop=(k == K1 - 1))
                ht = h_p.tile([128, NT], F16, tag=f"h1_{m}")
                evac(py, bc1, ht)
                h1.append(ht)

            # ---- L2 / L3 ----
            hcur = h1
            for lname, wts in (("l2", w2), ("l3", w3)):
                # stats: s1 = -mean, s2 = +E[h^2]
                s1 = ps_s.tile([1, NT], F32, tag="s1")
                s2 = ps_s.tile([1, NT], F32, tag="s2")
                for k in range(4):
                    nc.tensor.matmul(s1[:, :], lhsT=onesn[:, :], rhs=hcur[k][:, :],
                                     start=(k == 0), stop=(k == 3))
                for k in range(4):
                    sq = sq_p.tile([128, NT], F16, tag="sq")
                    nc.vector.tensor_mul(sq[:, :], hcur[k][:, :], hcur[k][:, :])
                    nc.tensor.matmul(s2[:, :], lhsT=onesp[:, :], rhs=sq[:, :],
                                     start=(k == 0), stop=(k == 3))
                musq = st_p.tile([1, NT], F32, tag="musq")
                nc.scalar.square(musq[:, :], s1[:, :])
                varr = st_p.tile([1, NT], F32, tag="var")
                nc.vector.tensor_sub(varr[:, :], s2[:, :], musq[:, :])
                negmu = h_p.tile([1, NT], F16, tag=f"negmu_{lname}")
                nc.vector.tensor_copy(out=negmu[:, :], in_=s1[:, :])
                sig32 = st_p.tile([1, NT], F32, tag="sig32")
                nc.scalar.activation(sig32[:, :], varr[:, :], AF.Sqrt, bias=epsT[0:1, :])
                sig16 = h_p.tile([1, NT], F16, tag=f"sig16_{lname}")
                nc.vector.tensor_copy(out=sig16[:, :], in_=sig32[:, :])
                invs = st_p.tile([1, NT], F32, tag="invs")
                nc.vector.reciprocal(invs[:, :], sig32[:, :])
                bct = bcast(invs[0:1, :])
                hnew = []
                for m in range(4):
                    py = ps_y.tile([128, NT], F32, tag="py")
                    msl = slice(m * 128, (m + 1) * 128)
                    for k in range(4):
                        nc.tensor.matmul(py[:, :], lhsT=wts[k][:, msl], rhs=hcur[k][:, :],
                                         start=(k == 0), stop=False)
                    nc.tensor.matmul(py[:, :], lhsT=wts[4][:, msl], rhs=negmu[:, :],
                                     start=False, stop=False)
                    nc.tensor.matmul(py[:, :], lhsT=wts[5][:, msl], rhs=sig16[:, :],
                                     start=False, stop=True)
                    ht = h_p.tile([128, NT], F16, tag=f"h_{lname}_{m}")
                    evac(py, bct, ht)
                    hnew.append(ht)
                hcur = hnew

            # ---- L4 ----
            pq = ps_q.tile([1, NT], F32, tag="pq")
            for k in range(4):
                nc.tensor.matmul(pq[:, :], lhsT=wo[:, k:k + 1], rhs=hcur[k][:, :],
                                 start=(k == 0), stop=(k == 3))
            nc.scalar.activation(qrow[0:1, bs:bs + NT], pq[:, :], AF.Tanh, bias=boutT[:, :])

        nc.sync.dma_start(out=q_d[:, :], in_=qrow[:, :])


# ---------------- host side ----------------

def host_prep(x, a, g1, beta1, g2, beta2, g3, beta3,
              w1, b1, w2, b2, w3, b3, w_out, b_out):
    """Shared (replicated) tensors + full z arrays; returns dict pieces."""
    f16 = np.float16
    z = np.empty((x.shape[0], D), dtype=f16)
    np.multiply(x[:, :HALF], np.float32(1.0 / X_NORM), out=z[:, :HALF], casting="unsafe")
    np.multiply(x[:, HALF:], np.float32(1.0 / V_NORM), out=z[:, HALF:INPUT_DIM], casting="unsafe")
    z[:, INPUT_DIM:] = a.astype(f16)

    def fold(w, g, beta, b, sigma_first):
        wg = (w.astype(np.float64) * g.astype(np.float64)[None, :])
        rs = wg.sum(axis=1)
        c = w.astype(np.float64) @ beta.astype(np.float64) + b.astype(np.float64)
        out = np.empty((w.shape[1] + 2, w.shape[0]), dtype=f16)
        out[:w.shape[1]] = wg.T.astype(f16)
        # L1 device aug rows arrive as (sigma, -mu) -> weight rows (c, rs);
        # L2/L3 use separate (negmu, sigma) rhs -> weight rows (rs, c).
        first, second = (c, rs) if sigma_first else (rs, c)
        out[w.shape[1]] = first.astype(f16)
        out[w.shape[1] + 1] = second.astype(f16)
        return out

    w1a = fold(w1, g1, beta1, b1, True)
    w2a = fold(w2, g2, beta2, b2, False)
    w3a = fold(w3, g3, beta3, b3, False)
    wout = w_out.T.astype(f16)  # [H, 1]
    bout = float(b_out[0])
    ident = np.eye(128, dtype=np.float32)
    return z, w1a, w2a, w3a, wout, bout, ident


_NC_CACHE = {}


def kernel(**inputs):
    inputs = {k: np.asarray(v) for k, v in inputs.items()}
    z, w1a, w2a, w3a, wout, bout, ident = host_prep(**inputs)

    key = (round(bout, 10), BC)
    if key not in _NC_CACHE:
        _NC_CACHE[key] = build_nc(bout, BC)
    nc = _NC_CACHE[key]

    in_maps = []
    for c in range(NCORES):
        zc = z[c * BC:(c + 1) * BC]
        in_maps.append({
            "zr": np.ascontiguousarray(zc),
            "zt": np.ascontiguousarray(zc.T),
            "w1a": w1a, "w2a": w2a, "w3a": w3a, "wout": wout, "ident": ident,
        })

    res = run_bass_kernel_spmd(nc, in_maps, list(range(NCORES)))
    q = np.concatenate([res.results[c]["q"].reshape(BC, 1) for c in range(NCORES)],
                       axis=0).astype(np.float32)
    return q



# revision 18
# speedup vs baseline: 150.2065x; 150.2065x over previous
"""Trainium2 Bass kernel for nn_Critic (dense MLP critic, 4 layers + LayerNorms).

Strategy (pure data parallel over 8 NeuronCores):
  - batch B=32768 sharded 8x -> 4096 rows/core; weights replicated.
  - activations feature-major ([features on partitions, batch on free dim]) so
    the contraction dim of every matmul is the partition dim.
  - LayerNorm folded into the matmuls:
      y = LN(z; g, beta) @ W.T + b
        = invs[b]*( (W*g)z[:,b] - mu[b]*rowsum(W*g) ) + (W@beta + b)
    realized as: psum = (Wg)z + (-mu)(x)rs  (aug K-row),
    then h = tanh(invs (.) psum + c) with c = W@beta+b applied as the
    per-partition bias AP of the ACT tanh (no sigma row, no Sqrt on ACT ->
    ScalarE never leaves the tanh table set; zero ACT_TABLE_LOADs in loop).
  - L1 stats (mean/var over 2080 feats) via bn_stats on a batch-major copy zr.
  - L2/L3 stats batch-major: PE-transpose h into PSUM, bn_stats along free.
  - invs = rsqrt(var+eps) on DVE: quake-style bit-trick seed + 1 Newton step
    (validated: adds <1e-4 to the fp16-dominated error).
  - fp16 data everywhere (weights, activations), f32 PSUM/statistics.
"""

import os
import sys
import numpy as np

for _p in ("/opt/trn_rl_repo",):
    if os.path.isdir(_p) and _p not in sys.path:
        sys.path.append(_p)

from contextlib import ExitStack

import concourse.bass as bass  # noqa: E402
import concourse.tile as tile  # noqa: E402
from concourse import bacc, mybir  # noqa: E402
from concourse.bass_utils import run_bass_kernel_spmd  # noqa: E402

NCORES = 8
B = 32768
BC = B // NCORES  # rows per core
INPUT_DIM = 2048
HALF = INPUT_DIM // 2
N_ACTIONS = 32
D = INPUT_DIM + N_ACTIONS  # 2080
H = 512
NT = 512  # batch columns per tile
EPS = 1e-5
X_NORM = 50.0
V_NORM = 10.0

F16 = mybir.dt.float16
F32 = mybir.dt.float32
U32 = mybir.dt.uint32
AF = mybir.ActivationFunctionType
ALU = mybir.AluOpType

K1 = 17  # ceil(D/128); last chunk has 32 data rows + 2 aug rows (invs, -mu)
K1_LAST = D - 16 * 128  # 32
RSQRT_MAGIC = 0x5F3759DF + 1
# Aug-row convention: device writes (invs, -mu) as two adjacent rows (so the
# psum->sbuf copy is one 2-partition, offset-0 access: partition offsets must
# be 32-aligned on TRN2). The matching weight rows are (0, rowsum(W*g)).


def build_nc(bout: float, bc: int = BC):
    """Build + compile the per-core program. bc = rows per core."""
    ntiles = bc // NT
    assert ntiles * NT == bc

    nc = bacc.Bacc("TRN2", target_bir_lowering=False, debug=False,
                   num_devices=NCORES)

    zr_d = nc.dram_tensor("zr", [bc, D], F16, kind="ExternalInput").ap()
    zt_d = nc.dram_tensor("zt", [D, bc], F16, kind="ExternalInput").ap()
    w1_d = nc.dram_tensor("w1a", [D + 2, H], F16, kind="ExternalInput").ap()
    w2_d = nc.dram_tensor("w2a", [H + 2, H], F16, kind="ExternalInput").ap()
    w3_d = nc.dram_tensor("w3a", [H + 2, H], F16, kind="ExternalInput").ap()
    wo_d = nc.dram_tensor("wout", [H, 1], F16, kind="ExternalInput").ap()
    c1_d = nc.dram_tensor("c1", [128, 4], F32, kind="ExternalInput").ap()
    c2_d = nc.dram_tensor("c2", [128, 4], F32, kind="ExternalInput").ap()
    c3_d = nc.dram_tensor("c3", [128, 4], F32, kind="ExternalInput").ap()
    i32_d = nc.dram_tensor("ident32", [128, 128], F32, kind="ExternalInput").ap()
    i16_d = nc.dram_tensor("ident16", [128, 128], F16, kind="ExternalInput").ap()
    q_d = nc.dram_tensor("q", [1, bc], F32, kind="ExternalOutput").ap()

    with tile.TileContext(nc) as tc:
        _emit(tc, ntiles, bout, zr_d, zt_d, w1_d, w2_d, w3_d, wo_d,
              c1_d, c2_d, c3_d, i32_d, i16_d, q_d)

    nc.compile()
    return nc


def _emit(tc, ntiles, bout, zr_d, zt_d, w1_d, w2_d, w3_d, wo_d,
          c1_d, c2_d, c3_d, i32_d, i16_d, q_d, dbg=None):
    nc = tc.nc
    with ExitStack() as ctx:
        wp = ctx.enter_context(tc.tile_pool(name="wp", bufs=1))
        zt_p = ctx.enter_context(tc.tile_pool(name="ztp", bufs=2))
        zr_p = ctx.enter_context(tc.tile_pool(name="zrp", bufs=2))
        h_p = ctx.enter_context(tc.tile_pool(name="hp", bufs=2))
        u_p = ctx.enter_context(tc.tile_pool(name="up", bufs=3))
        bc_p = ctx.enter_context(tc.tile_pool(name="bcp", bufs=2))
        st_p = ctx.enter_context(tc.tile_pool(name="stp", bufs=2))
        ps_y = ctx.enter_context(tc.tile_pool(name="psy", bufs=5, space="PSUM"))
        ps_tr = ctx.enter_context(tc.tile_pool(name="pstr", bufs=2, space="PSUM"))
        ps_q = ctx.enter_context(tc.tile_pool(name="psq", bufs=1, space="PSUM"))

        # ---- persistent constants / weights ----
        w1 = []
        for k in range(K1):
            rows = 128 if k < 16 else K1_LAST + 2
            t = wp.tile([rows, H], F16, tag=f"w1_{k}")
            nc.sync.dma_start(out=t[:, :], in_=w1_d[k * 128:k * 128 + rows, :])
            w1.append(t)
        w2 = []
        w3 = []
        aug2 = aug3 = None
        for name, wd, lst in (("w2", w2_d, w2), ("w3", w3_d, w3)):
            for k in range(4):
                t = wp.tile([128, H], F16, tag=f"{name}_{k}")
                nc.sync.dma_start(out=t[:, :], in_=wd[k * 128:(k + 1) * 128, :])
                lst.append(t)
            t = wp.tile([2, H], F16, tag=f"{name}_aug")
            nc.sync.dma_start(out=t[:, :], in_=wd[H:H + 2, :])
            if name == "w2":
                aug2 = t
            else:
                aug3 = t
        wo = wp.tile([128, 4], F16, tag="wo")
        for k in range(4):
            nc.sync.dma_start(out=wo[:, k:k + 1], in_=wo_d[k * 128:(k + 1) * 128, :])
        c1 = wp.tile([128, 4], F32, tag="c1")
        nc.sync.dma_start(out=c1[:, :], in_=c1_d[:, :])
        c2 = wp.tile([128, 4], F32, tag="c2")
        nc.sync.dma_start(out=c2[:, :], in_=c2_d[:, :])
        c3 = wp.tile([128, 4], F32, tag="c3")
        nc.sync.dma_start(out=c3[:, :], in_=c3_d[:, :])
        id32 = wp.tile([128, 128], F32, tag="id32")
        nc.sync.dma_start(out=id32[:, :], in_=i32_d[:, :])
        id16 = wp.tile([128, 128], F16, tag="id16")
        nc.sync.dma_start(out=id16[:, :], in_=i16_d[:, :])
        boutT = wp.tile([1, 1], F32, tag="boutT")
        nc.vector.memset(boutT[:, :], bout)
        qrow = wp.tile([1, ntiles * NT], F32, tag="qrow")

        def stats_to_pt(mv4, ptall):
            """From mv4 [128,4,2] (mean,var) build ptall [128,4,2] =
            (invs, -mu) per chunk, invs = rsqrt(var+EPS) via bit-trick +
            one Newton step (all on DVE, no ACT table switch)."""
            nc.vector.tensor_scalar_mul(ptall[:, :, 1], mv4[:, :, 0], -1.0)
            t4 = st_p.tile([128, 4], F32, tag="t4")
            nc.vector.tensor_scalar_add(t4[:, :], mv4[:, :, 1], EPS)
            y = ptall[:, :, 0]
            # seed bits = MAGIC - (bits(t) >> 1), computed in the fp32 ALU
            # (DVE int add is fp-rounded; shift is exact; the ~2^6-ulp seed
            # rounding is irrelevant after a Newton step). shf holds the
            # shifted bit pattern as an fp32 VALUE via dtype conversion.
            shf = st_p.tile([128, 4], F32, tag="shf")
            nc.vector.tensor_scalar(
                out=shf[:, :].bitcast(U32), in0=t4[:, :].bitcast(U32),
                scalar1=1, scalar2=None, op0=ALU.logical_shift_right)
            nc.vector.tensor_scalar(
                out=y.bitcast(U32), in0=shf[:, :].bitcast(U32), scalar1=-1.0,
                scalar2=float(RSQRT_MAGIC - 1), op0=ALU.mult, op1=ALU.add)
            tmp = st_p.tile([128, 4], F32, tag="nt_tmp")
            nc.vector.tensor_mul(tmp[:, :], y, y)
            nc.vector.tensor_mul(tmp[:, :], tmp[:, :], t4[:, :])
            nc.vector.tensor_scalar(
                out=tmp[:, :], in0=tmp[:, :], scalar1=-0.5, scalar2=1.5,
                op0=ALU.mult, op1=ALU.add)
            nc.vector.tensor_mul(y, y, tmp[:, :])

        for it in range(ntiles):
            bs = it * NT

            # ---- L1 stats: bn_stats over batch-major z rows ----
            zt16 = zt_p.tile([K1_LAST + 2, NT], F16, tag="zt16")
            zrt = zr_p.tile([128, 4, D], F16, tag="zrall")
            mv4 = st_p.tile([128, 4, 2], F32, tag="mv4")
            for bch in range(4):
                nc.sync.dma_start(out=zrt[:, bch, :],
                                  in_=zr_d[bs + bch * 128:bs + (bch + 1) * 128, :])
                stats = st_p.tile([128, 5, 6], F32, tag=f"st{bch}")
                zrv = zrt[:, bch, :].rearrange("p (n s) -> p n s", n=5)
                for i in range(5):
                    nc.vector.bn_stats(out=stats[:, i, :], in_=zrv[:, i, :])
                nc.vector.bn_aggr(out=mv4[:, bch, :], in_=stats[:, :, :])
            pt1 = st_p.tile([128, 4, 2], F32, tag="pt1")
            stats_to_pt(mv4, pt1)
            i1row = st_p.tile([1, NT], F16, tag="i1row")
            for c in range(4):
                sl = slice(c * 128, (c + 1) * 128)
                ptrc = ps_tr.tile([2, 128], F32, tag="tr")
                nc.tensor.transpose(out=ptrc[:, :], in_=pt1[:, c, :],
                                    identity=id32[:, :])
                nc.vector.tensor_copy(out=zt16[K1_LAST:K1_LAST + 2, sl],
                                      in_=ptrc[0:2, :])
                nc.vector.tensor_copy(out=i1row[0:1, sl], in_=ptrc[0:1, :])
            bc1 = bc_p.tile([128, NT], F16, tag="bc1")
            nc.gpsimd.partition_broadcast(bc1[:, :], i1row[0:1, :])

            # ---- zT loads: 4 DMAs of 4 chunks so L1 can start early ----
            ztmain = zt_p.tile([128, 16, NT], F16, tag="ztmain")
            for g in range(4):
                nc.sync.dma_start(
                    out=ztmain[:, g * 4:(g + 1) * 4, :],
                    in_=zt_d[g * 512:(g + 1) * 512, bs:bs + NT].rearrange(
                        "(k p) n -> p k n", k=4))
            nc.sync.dma_start(out=zt16[0:K1_LAST, :], in_=zt_d[2048:2048 + K1_LAST, bs:bs + NT])
            zts = [ztmain[:, k, :] for k in range(16)]

            # ---- L1 matmuls + evac ----
            h1 = []
            for m in range(4):
                py = ps_y.tile([128, NT], F32, tag="py")
                msl = slice(m * 128, (m + 1) * 128)
                for k in range(16):
                    nc.tensor.matmul(py[:, :], lhsT=w1[k][:, msl], rhs=zts[k],
                                     start=(k == 0), stop=False)
                nc.tensor.matmul(py[:, :], lhsT=w1[16][:, msl], rhs=zt16[:, :],
                                 start=False, stop=True)
                u = u_p.tile([128, NT], F16, tag="u")
                nc.vector.tensor_mul(u[:, :], py[:, :], bc1[:, :])
                ht = h_p.tile([128, NT], F16, tag=f"h1_{m}")
                nc.scalar.activation(ht[:, :], u[:, :], AF.Tanh, bias=c1[:, m:m + 1])
                h1.append(ht)
            if dbg is not None and it == 0:
                nc.sync.dma_start(out=dbg["zt16"], in_=zt16[:, :])
                nc.sync.dma_start(out=dbg["bc1"], in_=bc1[:, :])
                for m in range(4):
                    nc.sync.dma_start(out=dbg["h1"][m * 128:(m + 1) * 128, :],
                                      in_=h1[m][:, :])

            # ---- L2 / L3 ----
            hcur = h1
            for lname, wts, augr, cv in (("l2", w2, aug2, c2), ("l3", w3, aug3, c3)):
                # batch-major stats: PE-transpose h into PSUM, bn_stats there
                st6 = st_p.tile([128, 4, 6], F32, tag=f"st6_{lname}")
                mv4b = st_p.tile([128, 4, 2], F32, tag=f"mv4_{lname}")
                for j in range(4):
                    jsl = slice(j * 128, (j + 1) * 128)
                    trj = ps_tr.tile([128, 512], F16, tag="tr")
                    for k in range(4):
                        nc.tensor.transpose(out=trj[:, k * 128:(k + 1) * 128],
                                            in_=hcur[k][:, jsl],
                                            identity=id16[:, :])
                    nc.vector.bn_stats(out=st6[:, j, :], in_=trj[:, :])
                    nc.vector.bn_aggr(out=mv4b[:, j, :], in_=st6[:, j, :])
                ptl = st_p.tile([128, 4, 2], F32, tag=f"pt_{lname}")
                stats_to_pt(mv4b, ptl)
                ivnm = h_p.tile([2, NT], F16, tag=f"ivnm_{lname}")
                for c in range(4):
                    sl = slice(c * 128, (c + 1) * 128)
                    ptrc = ps_tr.tile([2, 128], F32, tag="tr")
                    nc.tensor.transpose(out=ptrc[:, :], in_=ptl[:, c, :],
                                        identity=id32[:, :])
                    nc.vector.tensor_copy(out=ivnm[0:2, sl], in_=ptrc[0:2, :])
                bct = bc_p.tile([128, NT], F16, tag=f"bc_{lname}")
                nc.gpsimd.partition_broadcast(bct[:, :], ivnm[0:1, :])

                hnew = []
                for m in range(4):
                    py = ps_y.tile([128, NT], F32, tag="py")
                    msl = slice(m * 128, (m + 1) * 128)
                    for k in range(4):
                        nc.tensor.matmul(py[:, :], lhsT=wts[k][:, msl], rhs=hcur[k][:, :],
                                         start=(k == 0), stop=False)
                    nc.tensor.matmul(py[:, :], lhsT=augr[:, msl], rhs=ivnm[:, :],
                                     start=False, stop=True)
                    u = u_p.tile([128, NT], F16, tag="u")
                    nc.vector.tensor_mul(u[:, :], py[:, :], bct[:, :])
                    ht = h_p.tile([128, NT], F16, tag=f"h_{lname}_{m}")
                    nc.scalar.activation(ht[:, :], u[:, :], AF.Tanh, bias=cv[:, m:m + 1])
                    hnew.append(ht)
                if dbg is not None and it == 0 and lname == "l2":
                    nc.sync.dma_start(out=dbg["ivnm2"], in_=ivnm[:, :])
                    for m in range(4):
                        nc.sync.dma_start(out=dbg["h2"][m * 128:(m + 1) * 128, :],
                                          in_=hnew[m][:, :])
                hcur = hnew

            # ---- L4 ----
            pq = ps_q.tile([1, NT], F32, tag="pq")
            for k in range(4):
                nc.tensor.matmul(pq[:, :], lhsT=wo[:, k:k + 1], rhs=hcur[k][:, :],
                                 start=(k == 0), stop=(k == 3))
            nc.scalar.activation(qrow[0:1, bs:bs + NT], pq[:, :], AF.Tanh, bias=boutT[:, :])

        nc.sync.dma_start(out=q_d[:, :], in_=qrow[:, :])


# ---------------- host side ----------------

def host_prep(x, a, g1, beta1, g2, beta2, g3, beta3,
              w1, b1, w2, b2, w3, b3, w_out, b_out):
    """Shared (replicated) tensors + full z arrays; returns dict pieces."""
    f16 = np.float16
    z = np.empty((x.shape[0], D), dtype=f16)
    np.multiply(x[:, :HALF], np.float32(1.0 / X_NORM), out=z[:, :HALF], casting="unsafe")
    np.multiply(x[:, HALF:], np.float32(1.0 / V_NORM), out=z[:, HALF:INPUT_DIM], casting="unsafe")
    z[:, INPUT_DIM:] = a.astype(f16)

    def fold(w, g, beta, b):
        wg = (w.astype(np.float64) * g.astype(np.float64)[None, :])
        rs = wg.sum(axis=1)
        c = w.astype(np.float64) @ beta.astype(np.float64) + b.astype(np.float64)
        out = np.empty((w.shape[1] + 2, w.shape[0]), dtype=f16)
        out[:w.shape[1]] = wg.T.astype(f16)
        # device aug rows arrive as (invs, -mu) -> weight rows (0, rs)
        out[w.shape[1]] = 0.0
        out[w.shape[1] + 1] = rs.astype(f16)
        cdev = np.ascontiguousarray(
            c.astype(np.float32).reshape(4, 128).T)  # [128, 4]
        return out, cdev

    w1a, c1 = fold(w1, g1, beta1, b1)
    w2a, c2 = fold(w2, g2, beta2, b2)
    w3a, c3 = fold(w3, g3, beta3, b3)
    wout = w_out.T.astype(f16)  # [H, 1]
    bout = float(b_out[0])
    ident32 = np.eye(128, dtype=np.float32)
    ident16 = np.eye(128, dtype=f16)
    return z, w1a, w2a, w3a, wout, bout, c1, c2, c3, ident32, ident16


def build_in_maps(inputs):
    z, w1a, w2a, w3a, wout, bout, c1, c2, c3, id32, id16 = host_prep(**inputs)
    in_maps = []
    for c in range(NCORES):
        zc = z[c * BC:(c + 1) * BC]
        in_maps.append({
            "zr": np.ascontiguousarray(zc),
            "zt": np.ascontiguousarray(zc.T),
            "w1a": w1a, "w2a": w2a, "w3a": w3a, "wout": wout,
            "c1": c1, "c2": c2, "c3": c3,
            "ident32": id32, "ident16": id16,
        })
    return in_maps, bout


_NC_CACHE = {}


def get_nc(bout: float):
    key = (round(bout, 10), BC)
    if key not in _NC_CACHE:
        _NC_CACHE[key] = build_nc(bout, BC)
    return _NC_CACHE[key]


def kernel(**inputs):
    inputs = {k: np.asarray(v) for k, v in inputs.items()}
    in_maps, bout = build_in_maps(inputs)
    nc = get_nc(bout)
    res = run_bass_kernel_spmd(nc, in_maps, list(range(NCORES)))
    q = np.concatenate([res.results[c]["q"].reshape(BC, 1) for c in range(NCORES)],
                       axis=0).astype(np.float32)
    return q


# revision 21
# speedup vs baseline: 161.8615x; 1.0776x over previous
"""Trainium2 Bass kernel for nn_Critic (dense MLP critic, 4 layers + LayerNorms).

Strategy (pure data parallel over 8 NeuronCores):
  - batch B=32768 sharded 8x -> 4096 rows/core; weights replicated.
  - activations feature-major ([features on partitions, batch on free dim]) so
    the contraction dim of every matmul is the partition dim.
  - LayerNorm folded into the matmuls:
      y = LN(z; g, beta) @ W.T + b
        = invs[b]*( (W*g)z[:,b] - mu[b]*rowsum(W*g) ) + (W@beta + b)
    realized as: psum = (Wg)z + (-mu)(x)rs  (aug K-row),
    then h = tanh(invs (.) psum + c) with c = W@beta+b applied as the
    per-partition bias AP of the ACT tanh (no sigma row, no Sqrt on ACT ->
    ScalarE never leaves the tanh table set; zero ACT_TABLE_LOADs in loop).
  - L1 stats (mean/var over 2080 feats) via bn_stats on a batch-major copy zr.
  - L2/L3 stats batch-major: PE-transpose h into PSUM, bn_stats along free.
  - invs = rsqrt(var+eps) on DVE: quake-style bit-trick seed + 1 Newton step
    (validated: adds <1e-4 to the fp16-dominated error).
  - fp16 data everywhere (weights, activations), f32 PSUM/statistics.
"""

import os
import sys
import numpy as np

for _p in ("/opt/trn_rl_repo",):
    if os.path.isdir(_p) and _p not in sys.path:
        sys.path.append(_p)

from contextlib import ExitStack

import concourse.bass as bass  # noqa: E402
import concourse.tile as tile  # noqa: E402
from concourse import bacc, mybir  # noqa: E402
from concourse.bass_utils import run_bass_kernel_spmd  # noqa: E402

NCORES = 8
B = 32768
BC = B // NCORES  # rows per core
INPUT_DIM = 2048
HALF = INPUT_DIM // 2
N_ACTIONS = 32
D = INPUT_DIM + N_ACTIONS  # 2080
H = 512
NT = 512  # batch columns per tile
EPS = 1e-5
X_NORM = 50.0
V_NORM = 10.0

F16 = mybir.dt.float16
F32 = mybir.dt.float32
U32 = mybir.dt.uint32
AF = mybir.ActivationFunctionType
ALU = mybir.AluOpType

K1 = 17  # ceil(D/128); last chunk has 32 data rows + 2 aug rows (invs, -mu)
K1_LAST = D - 16 * 128  # 32
RSQRT_MAGIC = 0x5F3759DF + 1
# Aug-row convention: device writes (invs, -mu) as two adjacent rows (so the
# psum->sbuf copy is one 2-partition, offset-0 access: partition offsets must
# be 32-aligned on TRN2). The matching weight rows are (0, rowsum(W*g)).


def build_nc(bout: float, bc: int = BC):
    """Build + compile the per-core program. bc = rows per core."""
    ntiles = bc // NT
    assert ntiles * NT == bc

    nc = bacc.Bacc("TRN2", target_bir_lowering=False, debug=False,
                   num_devices=NCORES)

    zr_d = nc.dram_tensor("zr", [bc, D], F16, kind="ExternalInput").ap()
    zt_d = nc.dram_tensor("zt", [D, bc], F16, kind="ExternalInput").ap()
    w1_d = nc.dram_tensor("w1a", [D + 2, H], F16, kind="ExternalInput").ap()
    w2_d = nc.dram_tensor("w2a", [H + 2, H], F16, kind="ExternalInput").ap()
    w3_d = nc.dram_tensor("w3a", [H + 2, H], F16, kind="ExternalInput").ap()
    wo_d = nc.dram_tensor("wout", [H, 1], F16, kind="ExternalInput").ap()
    c1_d = nc.dram_tensor("c1", [128, 4], F32, kind="ExternalInput").ap()
    c2_d = nc.dram_tensor("c2", [128, 4], F32, kind="ExternalInput").ap()
    c3_d = nc.dram_tensor("c3", [128, 4], F32, kind="ExternalInput").ap()
    i32_d = nc.dram_tensor("ident32", [128, 128], F32, kind="ExternalInput").ap()
    i16_d = nc.dram_tensor("ident16", [128, 128], F16, kind="ExternalInput").ap()
    q_d = nc.dram_tensor("q", [1, bc], F32, kind="ExternalOutput").ap()

    with tile.TileContext(nc) as tc:
        _emit(tc, ntiles, bout, zr_d, zt_d, w1_d, w2_d, w3_d, wo_d,
              c1_d, c2_d, c3_d, i32_d, i16_d, q_d)

    nc.compile()
    return nc


def _emit(tc, ntiles, bout, zr_d, zt_d, w1_d, w2_d, w3_d, wo_d,
          c1_d, c2_d, c3_d, i32_d, i16_d, q_d, dbg=None):
    nc = tc.nc
    with ExitStack() as ctx:
        wp = ctx.enter_context(tc.tile_pool(name="wp", bufs=1))
        zt_p = ctx.enter_context(tc.tile_pool(name="ztp", bufs=2))
        zr_p = ctx.enter_context(tc.tile_pool(name="zrp", bufs=2))
        h_p = ctx.enter_context(tc.tile_pool(name="hp", bufs=2))
        u_p = ctx.enter_context(tc.tile_pool(name="up", bufs=3))
        bc_p = ctx.enter_context(tc.tile_pool(name="bcp", bufs=2))
        st_p = ctx.enter_context(tc.tile_pool(name="stp", bufs=2))
        ps_y = ctx.enter_context(tc.tile_pool(name="psy", bufs=5, space="PSUM"))
        ps_tr = ctx.enter_context(tc.tile_pool(name="pstr", bufs=2, space="PSUM"))
        ps_q = ctx.enter_context(tc.tile_pool(name="psq", bufs=1, space="PSUM"))

        # ---- persistent constants / weights ----
        w1 = []
        for k in range(K1):
            rows = 128 if k < 16 else K1_LAST + 2
            t = wp.tile([rows, H], F16, tag=f"w1_{k}")
            nc.sync.dma_start(out=t[:, :], in_=w1_d[k * 128:k * 128 + rows, :])
            w1.append(t)
        w2 = []
        w3 = []
        aug2 = aug3 = None
        for name, wd, lst in (("w2", w2_d, w2), ("w3", w3_d, w3)):
            for k in range(4):
                t = wp.tile([128, H], F16, tag=f"{name}_{k}")
                nc.sync.dma_start(out=t[:, :], in_=wd[k * 128:(k + 1) * 128, :])
                lst.append(t)
            t = wp.tile([2, H], F16, tag=f"{name}_aug")
            nc.sync.dma_start(out=t[:, :], in_=wd[H:H + 2, :])
            if name == "w2":
                aug2 = t
            else:
                aug3 = t
        wo = wp.tile([128, 4], F16, tag="wo")
        for k in range(4):
            nc.sync.dma_start(out=wo[:, k:k + 1], in_=wo_d[k * 128:(k + 1) * 128, :])
        c1 = wp.tile([128, 4], F32, tag="c1")
        nc.sync.dma_start(out=c1[:, :], in_=c1_d[:, :])
        c2 = wp.tile([128, 4], F32, tag="c2")
        nc.sync.dma_start(out=c2[:, :], in_=c2_d[:, :])
        c3 = wp.tile([128, 4], F32, tag="c3")
        nc.sync.dma_start(out=c3[:, :], in_=c3_d[:, :])
        id32 = wp.tile([128, 128], F32, tag="id32")
        nc.sync.dma_start(out=id32[:, :], in_=i32_d[:, :])
        id16 = wp.tile([128, 128], F16, tag="id16")
        nc.sync.dma_start(out=id16[:, :], in_=i16_d[:, :])
        boutT = wp.tile([1, 1], F32, tag="boutT")
        nc.vector.memset(boutT[:, :], bout)
        qrow = wp.tile([1, ntiles * NT], F32, tag="qrow")

        def newton_rsqrt(t4, ptall):
            """ptall[:,:,0] = rsqrt(t4) via bit-trick seed + one Newton
            step (all on DVE, no ACT table switch). t4 = var + EPS."""
            y = ptall[:, :, 0]
            # seed bits = MAGIC - (bits(t) >> 1), computed in the fp32 ALU
            # (DVE int add is fp-rounded; shift is exact; the ~2^6-ulp seed
            # rounding is irrelevant after a Newton step). shf holds the
            # shifted bit pattern as an fp32 VALUE via dtype conversion.
            shf = st_p.tile([128, 4], F32, tag="shf")
            nc.vector.tensor_scalar(
                out=shf[:, :].bitcast(U32), in0=t4[:, :].bitcast(U32),
                scalar1=1, scalar2=None, op0=ALU.logical_shift_right)
            nc.vector.tensor_scalar(
                out=y.bitcast(U32), in0=shf[:, :].bitcast(U32), scalar1=-1.0,
                scalar2=float(RSQRT_MAGIC - 1), op0=ALU.mult, op1=ALU.add)
            tmp = st_p.tile([128, 4], F32, tag="nt_tmp")
            nc.vector.tensor_mul(tmp[:, :], y, y)
            nc.vector.tensor_mul(tmp[:, :], tmp[:, :], t4[:, :])
            nc.vector.tensor_scalar(
                out=tmp[:, :], in0=tmp[:, :], scalar1=-0.5, scalar2=1.5,
                op0=ALU.mult, op1=ALU.add)
            nc.vector.tensor_mul(y, y, tmp[:, :])

        def stats_to_pt(mv4, ptall):
            """From mv4 [128,4,2] (mean,var) build ptall [128,4,2] =
            (invs, -mu)."""
            nc.vector.tensor_scalar_mul(ptall[:, :, 1], mv4[:, :, 0], -1.0)
            t4 = st_p.tile([128, 4], F32, tag="t4")
            nc.vector.tensor_scalar_add(t4[:, :], mv4[:, :, 1], EPS)
            newton_rsqrt(t4, ptall)

        def load_z(it):
            """Allocate + DMA this tile's z data (zr batch-major chunks,
            zt feature-major)."""
            bs = it * NT
            zt16 = zt_p.tile([K1_LAST + 2, NT], F16, tag="zt16")
            zrt = zr_p.tile([128, 4, D], F16, tag="zrall")
            for bch in range(4):
                nc.sync.dma_start(out=zrt[:, bch, :],
                                  in_=zr_d[bs + bch * 128:bs + (bch + 1) * 128, :])
            ztmain = zt_p.tile([128, 16, NT], F16, tag="ztmain")
            for g in range(4):
                nc.sync.dma_start(
                    out=ztmain[:, g * 4:(g + 1) * 4, :],
                    in_=zt_d[g * 512:(g + 1) * 512, bs:bs + NT].rearrange(
                        "(k p) n -> p k n", k=4))
            nc.sync.dma_start(out=zt16[0:K1_LAST, :],
                              in_=zt_d[2048:2048 + K1_LAST, bs:bs + NT])
            return zrt, ztmain, zt16

        pre_z = load_z(0)

        for it in range(ntiles):
            bs = it * NT
            zrt, ztmain, zt16 = pre_z if it == 0 else load_z(it)
            zts = [ztmain[:, k, :] for k in range(16)]

            # ---- L1 stats: sum on ACT(Copy+accum), sumsq on ACT(Square+
            # accum) -> mean/var on DVE (keeps the in-order DVE free for
            # the latency-critical chain ops) ----
            s4 = st_p.tile([128, 4], F32, tag="s4")
            q4 = st_p.tile([128, 4], F32, tag="q4")
            for bch in range(4):
                scr = u_p.tile([128, D], F16, tag="scr")
                nc.scalar.activation(scr[:, :], zrt[:, bch, :], AF.Copy,
                                     accum_out=s4[:, bch:bch + 1])
                scr2 = u_p.tile([128, D], F16, tag="scr2")
                nc.scalar.activation(scr2[:, :], zrt[:, bch, :], AF.Square,
                                     accum_out=q4[:, bch:bch + 1])
            pt1 = st_p.tile([128, 4, 2], F32, tag="pt1")
            nc.vector.tensor_scalar_mul(pt1[:, :, 1], s4[:, :], -1.0 / D)
            m2 = st_p.tile([128, 4], F32, tag="m2")
            nc.vector.tensor_mul(m2[:, :], pt1[:, :, 1], pt1[:, :, 1])
            t4a = st_p.tile([128, 4], F32, tag="t4a")
            nc.vector.tensor_scalar(
                out=t4a[:, :], in0=q4[:, :], scalar1=1.0 / D, scalar2=EPS,
                op0=ALU.mult, op1=ALU.add)
            nc.vector.tensor_sub(t4a[:, :], t4a[:, :], m2[:, :])
            newton_rsqrt(t4a, pt1)
            i1row = st_p.tile([1, NT], F16, tag="i1row")
            for c in range(4):
                sl = slice(c * 128, (c + 1) * 128)
                ptrc = ps_tr.tile([2, 128], F32, tag="tr")
                nc.tensor.transpose(out=ptrc[:, :], in_=pt1[:, c, :],
                                    identity=id32[:, :])
                nc.vector.tensor_copy(out=zt16[K1_LAST:K1_LAST + 2, sl],
                                      in_=ptrc[0:2, :])
                nc.vector.tensor_copy(out=i1row[0:1, sl], in_=ptrc[0:1, :])
            bc1 = bc_p.tile([128, NT], F16, tag="bc1")
            nc.gpsimd.partition_broadcast(bc1[:, :], i1row[0:1, :])

            # ---- L1 matmuls + evac ----
            h1 = []
            for m in range(4):
                py = ps_y.tile([128, NT], F32, tag="py")
                msl = slice(m * 128, (m + 1) * 128)
                for k in range(16):
                    nc.tensor.matmul(py[:, :], lhsT=w1[k][:, msl], rhs=zts[k],
                                     start=(k == 0), stop=False)
                nc.tensor.matmul(py[:, :], lhsT=w1[16][:, msl], rhs=zt16[:, :],
                                 start=False, stop=True)
                u = u_p.tile([128, NT], F16, tag="u")
                nc.vector.tensor_mul(u[:, :], py[:, :], bc1[:, :])
                ht = h_p.tile([128, NT], F16, tag=f"h1_{m}")
                nc.scalar.activation(ht[:, :], u[:, :], AF.Tanh, bias=c1[:, m:m + 1])
                h1.append(ht)
            if dbg is not None and it == 0:
                nc.sync.dma_start(out=dbg["zt16"], in_=zt16[:, :])
                nc.sync.dma_start(out=dbg["bc1"], in_=bc1[:, :])
                for m in range(4):
                    nc.sync.dma_start(out=dbg["h1"][m * 128:(m + 1) * 128, :],
                                      in_=h1[m][:, :])

            # ---- L2 / L3 ----
            hcur = h1
            for lname, wts, augr, cv in (("l2", w2, aug2, c2), ("l3", w3, aug3, c3)):
                # batch-major stats: PE-transpose h into PSUM, bn_stats there
                st6 = st_p.tile([128, 4, 6], F32, tag=f"st6_{lname}")
                mv4b = st_p.tile([128, 4, 2], F32, tag=f"mv4_{lname}")
                for j in range(4):
                    jsl = slice(j * 128, (j + 1) * 128)
                    trj = ps_tr.tile([128, 512], F16, tag="tr")
                    for k in range(4):
                        nc.tensor.transpose(out=trj[:, k * 128:(k + 1) * 128],
                                            in_=hcur[k][:, jsl],
                                            identity=id16[:, :])
                    nc.vector.bn_stats(out=st6[:, j, :], in_=trj[:, :])
                    nc.vector.bn_aggr(out=mv4b[:, j, :], in_=st6[:, j, :])
                ptl = st_p.tile([128, 4, 2], F32, tag=f"pt_{lname}")
                stats_to_pt(mv4b, ptl)
                ivnm = h_p.tile([2, NT], F16, tag=f"ivnm_{lname}")
                for c in range(4):
                    sl = slice(c * 128, (c + 1) * 128)
                    ptrc = ps_tr.tile([2, 128], F32, tag="tr")
                    nc.tensor.transpose(out=ptrc[:, :], in_=ptl[:, c, :],
                                        identity=id32[:, :])
                    nc.vector.tensor_copy(out=ivnm[0:2, sl], in_=ptrc[0:2, :])
                bct = bc_p.tile([128, NT], F16, tag=f"bc_{lname}")
                nc.gpsimd.partition_broadcast(bct[:, :], ivnm[0:1, :])

                hnew = []
                for m in range(4):
                    py = ps_y.tile([128, NT], F32, tag="py")
                    msl = slice(m * 128, (m + 1) * 128)
                    for k in range(4):
                        nc.tensor.matmul(py[:, :], lhsT=wts[k][:, msl], rhs=hcur[k][:, :],
                                         start=(k == 0), stop=False)
                    nc.tensor.matmul(py[:, :], lhsT=augr[:, msl], rhs=ivnm[:, :],
                                     start=False, stop=True)
                    u = u_p.tile([128, NT], F16, tag="u")
                    nc.vector.tensor_mul(u[:, :], py[:, :], bct[:, :])
                    ht = h_p.tile([128, NT], F16, tag=f"h_{lname}_{m}")
                    nc.scalar.activation(ht[:, :], u[:, :], AF.Tanh, bias=cv[:, m:m + 1])
                    hnew.append(ht)
                if dbg is not None and it == 0 and lname == "l2":
                    nc.sync.dma_start(out=dbg["ivnm2"], in_=ivnm[:, :])
                    for m in range(4):
                        nc.sync.dma_start(out=dbg["h2"][m * 128:(m + 1) * 128, :],
                                          in_=hnew[m][:, :])
                hcur = hnew

            # ---- L4 ----
            pq = ps_q.tile([1, NT], F32, tag="pq")
            for k in range(4):
                nc.tensor.matmul(pq[:, :], lhsT=wo[:, k:k + 1], rhs=hcur[k][:, :],
                                 start=(k == 0), stop=(k == 3))
            nc.scalar.activation(qrow[0:1, bs:bs + NT], pq[:, :], AF.Tanh, bias=boutT[:, :])

        nc.sync.dma_start(out=q_d[:, :], in_=qrow[:, :])


# ---------------- host side ----------------

def host_prep(x, a, g1, beta1, g2, beta2, g3, beta3,
              w1, b1, w2, b2, w3, b3, w_out, b_out):
    """Shared (replicated) tensors + full z arrays; returns dict pieces."""
    f16 = np.float16
    z = np.empty((x.shape[0], D), dtype=f16)
    np.multiply(x[:, :HALF], np.float32(1.0 / X_NORM), out=z[:, :HALF], casting="unsafe")
    np.multiply(x[:, HALF:], np.float32(1.0 / V_NORM), out=z[:, HALF:INPUT_DIM], casting="unsafe")
    z[:, INPUT_DIM:] = a.astype(f16)

    def fold(w, g, beta, b):
        wg = (w.astype(np.float64) * g.astype(np.float64)[None, :])
        rs = wg.sum(axis=1)
        c = w.astype(np.float64) @ beta.astype(np.float64) + b.astype(np.float64)
        out = np.empty((w.shape[1] + 2, w.shape[0]), dtype=f16)
        out[:w.shape[1]] = wg.T.astype(f16)
        # device aug rows arrive as (invs, -mu) -> weight rows (0, rs)
        out[w.shape[1]] = 0.0
        out[w.shape[1] + 1] = rs.astype(f16)
        cdev = np.ascontiguousarray(
            c.astype(np.float32).reshape(4, 128).T)  # [128, 4]
        return out, cdev

    w1a, c1 = fold(w1, g1, beta1, b1)
    w2a, c2 = fold(w2, g2, beta2, b2)
    w3a, c3 = fold(w3, g3, beta3, b3)
    wout = w_out.T.astype(f16)  # [H, 1]
    bout = float(b_out[0])
    ident32 = np.eye(128, dtype=np.float32)
    ident16 = np.eye(128, dtype=f16)
    return z, w1a, w2a, w3a, wout, bout, c1, c2, c3, ident32, ident16


def build_in_maps(inputs):
    z, w1a, w2a, w3a, wout, bout, c1, c2, c3, id32, id16 = host_prep(**inputs)
    in_maps = []
    for c in range(NCORES):
        zc = z[c * BC:(c + 1) * BC]
        in_maps.append({
            "zr": np.ascontiguousarray(zc),
            "zt": np.ascontiguousarray(zc.T),
            "w1a": w1a, "w2a": w2a, "w3a": w3a, "wout": wout,
            "c1": c1, "c2": c2, "c3": c3,
            "ident32": id32, "ident16": id16,
        })
    return in_maps, bout


_NC_CACHE = {}


def get_nc(bout: float):
    key = (round(bout, 10), BC)
    if key not in _NC_CACHE:
        _NC_CACHE[key] = build_nc(bout, BC)
    return _NC_CACHE[key]


def kernel(**inputs):
    inputs = {k: np.asarray(v) for k, v in inputs.items()}
    in_maps, bout = build_in_maps(inputs)
    nc = get_nc(bout)
    res = run_bass_kernel_spmd(nc, in_maps, list(range(NCORES)))
    q = np.concatenate([res.results[c]["q"].reshape(BC, 1) for c in range(NCORES)],
                       axis=0).astype(np.float32)
    return q


# revision 23
# speedup vs baseline: 192.0465x; 1.1865x over previous
"""Trainium2 Bass kernel for nn_Critic (dense MLP critic, 4 layers + LayerNorms).

Strategy (pure data parallel over 8 NeuronCores):
  - batch B=32768 sharded 8x -> 4096 rows/core; weights replicated.
  - activations feature-major ([features on partitions, batch on free dim]) so
    the contraction dim of every matmul is the partition dim.
  - LayerNorm folded into the matmuls:
      y = LN(z; g, beta) @ W.T + b
        = invs[b]*( (W*g)z[:,b] - mu[b]*rowsum(W*g) ) + (W@beta + b)
    realized as: psum = (Wg)z + (-mu)(x)rs  (aug K-row),
    then h = tanh(invs (.) psum + c) with c = W@beta+b applied as the
    per-partition bias AP of the ACT tanh (no sigma row, no Sqrt on ACT ->
    ScalarE never leaves the tanh table set; zero ACT_TABLE_LOADs in loop).
  - L1 stats (mean/var over 2080 feats) via bn_stats on a batch-major copy zr.
  - L2/L3 stats batch-major: PE-transpose h into PSUM, bn_stats along free.
  - invs = rsqrt(var+eps) on DVE: quake-style bit-trick seed + 1 Newton step
    (validated: adds <1e-4 to the fp16-dominated error).
  - fp16 data everywhere (weights, activations), f32 PSUM/statistics.
"""

import os
import sys
import numpy as np

for _p in ("/opt/trn_rl_repo",):
    if os.path.isdir(_p) and _p not in sys.path:
        sys.path.append(_p)

from contextlib import ExitStack

import concourse.bass as bass  # noqa: E402
import concourse.tile as tile  # noqa: E402
from concourse import bacc, mybir  # noqa: E402
from concourse.bass_utils import run_bass_kernel_spmd  # noqa: E402

NCORES = 8
B = 32768
BC = B // NCORES  # rows per core
INPUT_DIM = 2048
HALF = INPUT_DIM // 2
N_ACTIONS = 32
D = INPUT_DIM + N_ACTIONS  # 2080
H = 512
NT = 512  # batch columns per tile
EPS = 1e-5
X_NORM = 50.0
V_NORM = 10.0

F16 = mybir.dt.float16
F32 = mybir.dt.float32
U32 = mybir.dt.uint32
AF = mybir.ActivationFunctionType
ALU = mybir.AluOpType

K1 = 17  # ceil(D/128); last chunk has 32 data rows + 2 aug rows (invs, -mu)
K1_LAST = D - 16 * 128  # 32
RSQRT_MAGIC = 0x5F3759DF + 1
# Aug-row convention: device writes (invs, -mu) as two adjacent rows (so the
# psum->sbuf copy is one 2-partition, offset-0 access: partition offsets must
# be 32-aligned on TRN2). The matching weight rows are (0, rowsum(W*g)).


def build_nc(bout: float, bc: int = BC):
    """Build + compile the per-core program. bc = rows per core."""
    ntiles = bc // NT
    assert ntiles * NT == bc

    nc = bacc.Bacc("TRN2", target_bir_lowering=False, debug=False,
                   num_devices=NCORES)

    zr_d = nc.dram_tensor("zr", [bc, D], F16, kind="ExternalInput").ap()
    zt_d = nc.dram_tensor("zt", [D, bc], F16, kind="ExternalInput").ap()
    w1_d = nc.dram_tensor("w1a", [D + 2, H], F16, kind="ExternalInput").ap()
    w2_d = nc.dram_tensor("w2a", [H + 2, H], F16, kind="ExternalInput").ap()
    w3_d = nc.dram_tensor("w3a", [H + 2, H], F16, kind="ExternalInput").ap()
    wo_d = nc.dram_tensor("wout", [H, 1], F16, kind="ExternalInput").ap()
    c1_d = nc.dram_tensor("c1", [128, 4], F32, kind="ExternalInput").ap()
    c2_d = nc.dram_tensor("c2", [128, 4], F32, kind="ExternalInput").ap()
    c3_d = nc.dram_tensor("c3", [128, 4], F32, kind="ExternalInput").ap()
    i32_d = nc.dram_tensor("ident32", [128, 128], F32, kind="ExternalInput").ap()
    i16_d = nc.dram_tensor("ident16", [128, 128], F16, kind="ExternalInput").ap()
    q_d = nc.dram_tensor("q", [1, bc], F32, kind="ExternalOutput").ap()

    with tile.TileContext(nc) as tc:
        _emit(tc, ntiles, bout, zr_d, zt_d, w1_d, w2_d, w3_d, wo_d,
              c1_d, c2_d, c3_d, i32_d, i16_d, q_d)

    nc.compile()
    return nc


def _emit(tc, ntiles, bout, zr_d, zt_d, w1_d, w2_d, w3_d, wo_d,
          c1_d, c2_d, c3_d, i32_d, i16_d, q_d, dbg=None):
    nc = tc.nc
    with ExitStack() as ctx:
        wp = ctx.enter_context(tc.tile_pool(name="wp", bufs=1))
        zt_p = ctx.enter_context(tc.tile_pool(name="ztp", bufs=2))
        zr_p = ctx.enter_context(tc.tile_pool(name="zrp", bufs=2))
        h_p = ctx.enter_context(tc.tile_pool(name="hp", bufs=2))
        u_p = ctx.enter_context(tc.tile_pool(name="up", bufs=3))
        bc_p = ctx.enter_context(tc.tile_pool(name="bcp", bufs=2))
        st_p = ctx.enter_context(tc.tile_pool(name="stp", bufs=2))
        ps_y = ctx.enter_context(tc.tile_pool(name="psy", bufs=5, space="PSUM"))
        ps_tr = ctx.enter_context(tc.tile_pool(name="pstr", bufs=2, space="PSUM"))
        ps_q = ctx.enter_context(tc.tile_pool(name="psq", bufs=1, space="PSUM"))

        # ---- persistent constants / weights ----
        w1 = []
        for k in range(K1):
            rows = 128 if k < 16 else K1_LAST + 2
            t = wp.tile([rows, H], F16, tag=f"w1_{k}")
            nc.sync.dma_start(out=t[:, :], in_=w1_d[k * 128:k * 128 + rows, :])
            w1.append(t)
        w2 = []
        w3 = []
        aug2 = aug3 = None
        for name, wd, lst in (("w2", w2_d, w2), ("w3", w3_d, w3)):
            for k in range(4):
                t = wp.tile([128, H], F16, tag=f"{name}_{k}")
                nc.sync.dma_start(out=t[:, :], in_=wd[k * 128:(k + 1) * 128, :])
                lst.append(t)
            t = wp.tile([2, H], F16, tag=f"{name}_aug")
            nc.sync.dma_start(out=t[:, :], in_=wd[H:H + 2, :])
            if name == "w2":
                aug2 = t
            else:
                aug3 = t
        wo = wp.tile([128, 4], F16, tag="wo")
        for k in range(4):
            nc.sync.dma_start(out=wo[:, k:k + 1], in_=wo_d[k * 128:(k + 1) * 128, :])
        c1 = wp.tile([128, 4], F32, tag="c1")
        nc.sync.dma_start(out=c1[:, :], in_=c1_d[:, :])
        c2 = wp.tile([128, 4], F32, tag="c2")
        nc.sync.dma_start(out=c2[:, :], in_=c2_d[:, :])
        c3 = wp.tile([128, 4], F32, tag="c3")
        nc.sync.dma_start(out=c3[:, :], in_=c3_d[:, :])
        id32 = wp.tile([128, 128], F32, tag="id32")
        nc.sync.dma_start(out=id32[:, :], in_=i32_d[:, :])
        id16 = wp.tile([128, 128], F16, tag="id16")
        nc.sync.dma_start(out=id16[:, :], in_=i16_d[:, :])
        boutT = wp.tile([1, 1], F32, tag="boutT")
        nc.vector.memset(boutT[:, :], bout)
        qrow = wp.tile([1, ntiles * NT], F32, tag="qrow")

        def newton_rsqrt(t4, ptall):
            """ptall[:,:,0] = rsqrt(t4) via bit-trick seed + one Newton
            step (all on DVE, no ACT table switch). t4 = var + EPS."""
            y = ptall[:, :, 0]
            # seed bits = MAGIC - (bits(t) >> 1), computed in the fp32 ALU
            # (DVE int add is fp-rounded; shift is exact; the ~2^6-ulp seed
            # rounding is irrelevant after a Newton step). shf holds the
            # shifted bit pattern as an fp32 VALUE via dtype conversion.
            shf = st_p.tile([128, 4], F32, tag="shf")
            nc.vector.tensor_scalar(
                out=shf[:, :].bitcast(U32), in0=t4[:, :].bitcast(U32),
                scalar1=1, scalar2=None, op0=ALU.logical_shift_right)
            nc.vector.tensor_scalar(
                out=y.bitcast(U32), in0=shf[:, :].bitcast(U32), scalar1=-1.0,
                scalar2=float(RSQRT_MAGIC - 1), op0=ALU.mult, op1=ALU.add)
            tmp = st_p.tile([128, 4], F32, tag="nt_tmp")
            nc.vector.tensor_mul(tmp[:, :], y, y)
            nc.vector.tensor_mul(tmp[:, :], tmp[:, :], t4[:, :])
            nc.vector.tensor_scalar(
                out=tmp[:, :], in0=tmp[:, :], scalar1=-0.5, scalar2=1.5,
                op0=ALU.mult, op1=ALU.add)
            nc.vector.tensor_mul(y, y, tmp[:, :])

        def stats_to_pt(mv4, ptall):
            """From mv4 [128,4,2] (mean,var) build ptall [128,4,2] =
            (invs, -mu)."""
            nc.vector.tensor_scalar_mul(ptall[:, :, 1], mv4[:, :, 0], -1.0)
            t4 = st_p.tile([128, 4], F32, tag="t4")
            nc.vector.tensor_scalar_add(t4[:, :], mv4[:, :, 1], EPS)
            newton_rsqrt(t4, ptall)

        def load_z(it):
            """Allocate + DMA this tile's z data. Separate tiles per DMA so
            readers only wait on their own chunk's transfer."""
            bs = it * NT
            zt16 = zt_p.tile([K1_LAST + 2, NT], F16, tag="zt16")
            zr4 = []
            for bch in range(4):
                zc = zr_p.tile([128, D], F16, tag=f"zr_{bch}")
                nc.sync.dma_start(out=zc[:, :],
                                  in_=zr_d[bs + bch * 128:bs + (bch + 1) * 128, :])
                zr4.append(zc)
            ztg = []
            for g in range(4):
                zg = zt_p.tile([128, 4, NT], F16, tag=f"ztg_{g}")
                nc.sync.dma_start(
                    out=zg[:, :, :],
                    in_=zt_d[g * 512:(g + 1) * 512, bs:bs + NT].rearrange(
                        "(k p) n -> p k n", k=4))
                ztg.append(zg)
            nc.sync.dma_start(out=zt16[0:K1_LAST, :],
                              in_=zt_d[2048:2048 + K1_LAST, bs:bs + NT])
            return zr4, ztg, zt16

        pre_z = load_z(0)

        for it in range(ntiles):
            bs = it * NT
            zr4, ztg, zt16 = pre_z if it == 0 else load_z(it)
            zts = [ztg[k // 4][:, k % 4, :] for k in range(16)]

            # ---- L1 stats: sum on ACT(Copy+accum), sumsq on ACT(Square+
            # accum) -> mean/var on DVE (keeps the in-order DVE free for
            # the latency-critical chain ops) ----
            s4 = st_p.tile([128, 4], F32, tag="s4")
            q4 = st_p.tile([128, 4], F32, tag="q4")
            for bch in range(4):
                scr = u_p.tile([128, D], F16, tag="scr")
                nc.scalar.activation(scr[:, :], zr4[bch][:, :], AF.Copy,
                                     accum_out=s4[:, bch:bch + 1])
                scr2 = u_p.tile([128, D], F16, tag="scr2")
                nc.scalar.activation(scr2[:, :], zr4[bch][:, :], AF.Square,
                                     accum_out=q4[:, bch:bch + 1])
            pt1 = st_p.tile([128, 4, 2], F32, tag="pt1")
            nc.vector.tensor_scalar_mul(pt1[:, :, 1], s4[:, :], -1.0 / D)
            m2 = st_p.tile([128, 4], F32, tag="m2")
            nc.vector.tensor_mul(m2[:, :], pt1[:, :, 1], pt1[:, :, 1])
            t4a = st_p.tile([128, 4], F32, tag="t4a")
            nc.vector.tensor_scalar(
                out=t4a[:, :], in0=q4[:, :], scalar1=1.0 / D, scalar2=EPS,
                op0=ALU.mult, op1=ALU.add)
            nc.vector.tensor_sub(t4a[:, :], t4a[:, :], m2[:, :])
            newton_rsqrt(t4a, pt1)
            i1row = st_p.tile([1, NT], F16, tag="i1row")
            for c in range(4):
                sl = slice(c * 128, (c + 1) * 128)
                ptrc = ps_tr.tile([2, 128], F32, tag="tr")
                nc.tensor.transpose(out=ptrc[:, :], in_=pt1[:, c, :],
                                    identity=id32[:, :])
                nc.vector.tensor_copy(out=zt16[K1_LAST:K1_LAST + 2, sl],
                                      in_=ptrc[0:2, :])
                nc.vector.tensor_copy(out=i1row[0:1, sl], in_=ptrc[0:1, :])
            bc1 = bc_p.tile([128, NT], F16, tag="bc1")
            nc.gpsimd.partition_broadcast(bc1[:, :], i1row[0:1, :])

            # ---- L1 matmuls + evac ----
            h1 = []
            for m in range(4):
                py = ps_y.tile([128, NT], F32, tag="py")
                msl = slice(m * 128, (m + 1) * 128)
                for k in range(16):
                    nc.tensor.matmul(py[:, :], lhsT=w1[k][:, msl], rhs=zts[k],
                                     start=(k == 0), stop=False)
                nc.tensor.matmul(py[:, :], lhsT=w1[16][:, msl], rhs=zt16[:, :],
                                 start=False, stop=True)
                u = u_p.tile([128, NT], F16, tag="u")
                nc.vector.tensor_mul(u[:, :], py[:, :], bc1[:, :])
                ht = h_p.tile([128, NT], F16, tag=f"h1_{m}")
                nc.scalar.activation(ht[:, :], u[:, :], AF.Tanh, bias=c1[:, m:m + 1])
                h1.append(ht)
            if dbg is not None and it == 0:
                nc.sync.dma_start(out=dbg["zt16"], in_=zt16[:, :])
                nc.sync.dma_start(out=dbg["bc1"], in_=bc1[:, :])
                for m in range(4):
                    nc.sync.dma_start(out=dbg["h1"][m * 128:(m + 1) * 128, :],
                                      in_=h1[m][:, :])

            # ---- L2 / L3 ----
            hcur = h1
            for lname, wts, augr, cv in (("l2", w2, aug2, c2), ("l3", w3, aug3, c3)):
                # batch-major stats: PE-transpose h into PSUM, bn_stats there
                st6 = st_p.tile([128, 4, 6], F32, tag=f"st6_{lname}")
                mv4b = st_p.tile([128, 4, 2], F32, tag=f"mv4_{lname}")
                for j in range(4):
                    jsl = slice(j * 128, (j + 1) * 128)
                    trj = ps_tr.tile([128, 512], F16, tag="tr")
                    for k in range(4):
                        nc.tensor.transpose(out=trj[:, k * 128:(k + 1) * 128],
                                            in_=hcur[k][:, jsl],
                                            identity=id16[:, :])
                    nc.vector.bn_stats(out=st6[:, j, :], in_=trj[:, :])
                    nc.vector.bn_aggr(out=mv4b[:, j, :], in_=st6[:, j, :])
                ptl = st_p.tile([128, 4, 2], F32, tag=f"pt_{lname}")
                stats_to_pt(mv4b, ptl)
                ivnm = h_p.tile([2, NT], F16, tag=f"ivnm_{lname}")
                for c in range(4):
                    sl = slice(c * 128, (c + 1) * 128)
                    ptrc = ps_tr.tile([2, 128], F32, tag="tr")
                    nc.tensor.transpose(out=ptrc[:, :], in_=ptl[:, c, :],
                                        identity=id32[:, :])
                    nc.vector.tensor_copy(out=ivnm[0:2, sl], in_=ptrc[0:2, :])
                bct = bc_p.tile([128, NT], F16, tag=f"bc_{lname}")
                nc.gpsimd.partition_broadcast(bct[:, :], ivnm[0:1, :])

                hnew = []
                for m in range(4):
                    py = ps_y.tile([128, NT], F32, tag="py")
                    msl = slice(m * 128, (m + 1) * 128)
                    for k in range(4):
                        nc.tensor.matmul(py[:, :], lhsT=wts[k][:, msl], rhs=hcur[k][:, :],
                                         start=(k == 0), stop=False)
                    nc.tensor.matmul(py[:, :], lhsT=augr[:, msl], rhs=ivnm[:, :],
                                     start=False, stop=True)
                    u = u_p.tile([128, NT], F16, tag="u")
                    nc.vector.tensor_mul(u[:, :], py[:, :], bct[:, :])
                    ht = h_p.tile([128, NT], F16, tag=f"h_{lname}_{m}")
                    nc.scalar.activation(ht[:, :], u[:, :], AF.Tanh, bias=cv[:, m:m + 1])
                    hnew.append(ht)
                if dbg is not None and it == 0 and lname == "l2":
                    nc.sync.dma_start(out=dbg["ivnm2"], in_=ivnm[:, :])
                    for m in range(4):
                        nc.sync.dma_start(out=dbg["h2"][m * 128:(m + 1) * 128, :],
                                          in_=hnew[m][:, :])
                hcur = hnew

            # ---- L4 ----
            pq = ps_q.tile([1, NT], F32, tag="pq")
            for k in range(4):
                nc.tensor.matmul(pq[:, :], lhsT=wo[:, k:k + 1], rhs=hcur[k][:, :],
                                 start=(k == 0), stop=(k == 3))
            nc.scalar.activation(qrow[0:1, bs:bs + NT], pq[:, :], AF.Tanh, bias=boutT[:, :])

        nc.sync.dma_start(out=q_d[:, :], in_=qrow[:, :])


# ---------------- host side ----------------

def host_prep(x, a, g1, beta1, g2, beta2, g3, beta3,
              w1, b1, w2, b2, w3, b3, w_out, b_out):
    """Shared (replicated) tensors + full z arrays; returns dict pieces."""
    f16 = np.float16
    z = np.empty((x.shape[0], D), dtype=f16)
    np.multiply(x[:, :HALF], np.float32(1.0 / X_NORM), out=z[:, :HALF], casting="unsafe")
    np.multiply(x[:, HALF:], np.float32(1.0 / V_NORM), out=z[:, HALF:INPUT_DIM], casting="unsafe")
    z[:, INPUT_DIM:] = a.astype(f16)

    def fold(w, g, beta, b):
        wg = (w.astype(np.float64) * g.astype(np.float64)[None, :])
        rs = wg.sum(axis=1)
        c = w.astype(np.float64) @ beta.astype(np.float64) + b.astype(np.float64)
        out = np.empty((w.shape[1] + 2, w.shape[0]), dtype=f16)
        out[:w.shape[1]] = wg.T.astype(f16)
        # device aug rows arrive as (invs, -mu) -> weight rows (0, rs)
        out[w.shape[1]] = 0.0
        out[w.shape[1] + 1] = rs.astype(f16)
        cdev = np.ascontiguousarray(
            c.astype(np.float32).reshape(4, 128).T)  # [128, 4]
        return out, cdev

    w1a, c1 = fold(w1, g1, beta1, b1)
    w2a, c2 = fold(w2, g2, beta2, b2)
    w3a, c3 = fold(w3, g3, beta3, b3)
    wout = w_out.T.astype(f16)  # [H, 1]
    bout = float(b_out[0])
    ident32 = np.eye(128, dtype=np.float32)
    ident16 = np.eye(128, dtype=f16)
    return z, w1a, w2a, w3a, wout, bout, c1, c2, c3, ident32, ident16


def build_in_maps(inputs):
    z, w1a, w2a, w3a, wout, bout, c1, c2, c3, id32, id16 = host_prep(**inputs)
    in_maps = []
    for c in range(NCORES):
        zc = z[c * BC:(c + 1) * BC]
        in_maps.append({
            "zr": np.ascontiguousarray(zc),
            "zt": np.ascontiguousarray(zc.T),
            "w1a": w1a, "w2a": w2a, "w3a": w3a, "wout": wout,
            "c1": c1, "c2": c2, "c3": c3,
            "ident32": id32, "ident16": id16,
        })
    return in_maps, bout


_NC_CACHE = {}


def get_nc(bout: float):
    key = (round(bout, 10), BC)
    if key not in _NC_CACHE:
        _NC_CACHE[key] = build_nc(bout, BC)
    return _NC_CACHE[key]


def kernel(**inputs):
    inputs = {k: np.asarray(v) for k, v in inputs.items()}
    in_maps, bout = build_in_maps(inputs)
    nc = get_nc(bout)
    res = run_bass_kernel_spmd(nc, in_maps, list(range(NCORES)))
    q = np.concatenate([res.results[c]["q"].reshape(BC, 1) for c in range(NCORES)],
                       axis=0).astype(np.float32)
    return q


# revision 26
# speedup vs baseline: 194.5154x; 1.0129x over previous
"""Trainium2 Bass kernel for nn_Critic (dense MLP critic, 4 layers + LayerNorms).

Strategy (pure data parallel over 8 NeuronCores):
  - batch B=32768 sharded 8x -> 4096 rows/core; weights replicated.
  - activations feature-major ([features on partitions, batch on free dim]) so
    the contraction dim of every matmul is the partition dim.
  - LayerNorm folded into the matmuls:
      y = LN(z; g, beta) @ W.T + b
        = invs[b]*( (W*g)z[:,b] - mu[b]*rowsum(W*g) ) + (W@beta + b)
    realized as: psum = (Wg)z + (-mu)(x)rs  (aug K-row),
    then h = tanh(invs (.) psum + c) with c = W@beta+b applied as the
    per-partition bias AP of the ACT tanh (no sigma row, no Sqrt on ACT ->
    ScalarE never leaves the tanh table set; zero ACT_TABLE_LOADs in loop).
  - L1 stats (mean/var over 2080 feats) via bn_stats on a batch-major copy zr.
  - L2/L3 stats batch-major: PE-transpose h into PSUM, bn_stats along free.
  - invs = rsqrt(var+eps) on DVE: quake-style bit-trick seed + 1 Newton step
    (validated: adds <1e-4 to the fp16-dominated error).
  - fp16 data everywhere (weights, activations), f32 PSUM/statistics.
"""

import os
import sys
import numpy as np

for _p in ("/opt/trn_rl_repo",):
    if os.path.isdir(_p) and _p not in sys.path:
        sys.path.append(_p)

from contextlib import ExitStack

import concourse.bass as bass  # noqa: E402
import concourse.tile as tile  # noqa: E402
from concourse import bacc, mybir  # noqa: E402
from concourse.bass_utils import run_bass_kernel_spmd  # noqa: E402

NCORES = 8
B = 32768
BC = B // NCORES  # rows per core
INPUT_DIM = 2048
HALF = INPUT_DIM // 2
N_ACTIONS = 32
D = INPUT_DIM + N_ACTIONS  # 2080
H = 512
NT = 512  # batch columns per tile
EPS = 1e-5
X_NORM = 50.0
V_NORM = 10.0

F16 = mybir.dt.float16
F32 = mybir.dt.float32
U32 = mybir.dt.uint32
AF = mybir.ActivationFunctionType
ALU = mybir.AluOpType

K1 = 17  # ceil(D/128); last chunk has 32 data rows + 2 aug rows (invs, -mu)
K1_LAST = D - 16 * 128  # 32
RSQRT_MAGIC = 0x5F3759DF + 1
# Aug-row convention: device writes (invs, -mu) as two adjacent rows (so the
# psum->sbuf copy is one 2-partition, offset-0 access: partition offsets must
# be 32-aligned on TRN2). The matching weight rows are (0, rowsum(W*g)).


def build_nc(bout: float, bc: int = BC):
    """Build + compile the per-core program. bc = rows per core."""
    ntiles = bc // NT
    assert ntiles * NT == bc

    nc = bacc.Bacc("TRN2", target_bir_lowering=False, debug=False,
                   num_devices=NCORES)

    zr_d = nc.dram_tensor("zr", [bc, D], F16, kind="ExternalInput").ap()
    zt_d = nc.dram_tensor("zt", [D, bc], F16, kind="ExternalInput").ap()
    w1_d = nc.dram_tensor("w1a", [D + 2, H], F16, kind="ExternalInput").ap()
    w2_d = nc.dram_tensor("w2a", [H + 2, H], F16, kind="ExternalInput").ap()
    w3_d = nc.dram_tensor("w3a", [H + 2, H], F16, kind="ExternalInput").ap()
    wo_d = nc.dram_tensor("wout", [H, 1], F16, kind="ExternalInput").ap()
    c1_d = nc.dram_tensor("c1", [128, 4], F32, kind="ExternalInput").ap()
    c2_d = nc.dram_tensor("c2", [128, 4], F32, kind="ExternalInput").ap()
    c3_d = nc.dram_tensor("c3", [128, 4], F32, kind="ExternalInput").ap()
    i32_d = nc.dram_tensor("ident32", [128, 128], F32, kind="ExternalInput").ap()
    i16_d = nc.dram_tensor("ident16", [128, 128], F16, kind="ExternalInput").ap()
    q_d = nc.dram_tensor("q", [1, bc], F32, kind="ExternalOutput").ap()

    with tile.TileContext(nc) as tc:
        _emit(tc, ntiles, bout, zr_d, zt_d, w1_d, w2_d, w3_d, wo_d,
              c1_d, c2_d, c3_d, i32_d, i16_d, q_d)

    nc.compile()
    return nc


def _emit(tc, ntiles, bout, zr_d, zt_d, w1_d, w2_d, w3_d, wo_d,
          c1_d, c2_d, c3_d, i32_d, i16_d, q_d, dbg=None):
    nc = tc.nc
    with ExitStack() as ctx:
        wp = ctx.enter_context(tc.tile_pool(name="wp", bufs=1))
        zt_p = ctx.enter_context(tc.tile_pool(name="ztp", bufs=2))
        zr_p = ctx.enter_context(tc.tile_pool(name="zrp", bufs=2))
        h_p = ctx.enter_context(tc.tile_pool(name="hp", bufs=2))
        u_p = ctx.enter_context(tc.tile_pool(name="up", bufs=3))
        bc_p = ctx.enter_context(tc.tile_pool(name="bcp", bufs=2))
        st_p = ctx.enter_context(tc.tile_pool(name="stp", bufs=2))
        ps_y = ctx.enter_context(tc.tile_pool(name="psy", bufs=5, space="PSUM"))
        ps_tr = ctx.enter_context(tc.tile_pool(name="pstr", bufs=2, space="PSUM"))
        ps_q = ctx.enter_context(tc.tile_pool(name="psq", bufs=1, space="PSUM"))

        def newton_rsqrt(t4, ptall):
            """ptall[:,:,0] = rsqrt(t4) via bit-trick seed + one Newton
            step (all on DVE, no ACT table switch). t4 = var + EPS."""
            y = ptall[:, :, 0]
            # seed bits = MAGIC - (bits(t) >> 1), computed in the fp32 ALU
            # (DVE int add is fp-rounded; shift is exact; the ~2^6-ulp seed
            # rounding is irrelevant after a Newton step). shf holds the
            # shifted bit pattern as an fp32 VALUE via dtype conversion.
            shf = st_p.tile([128, 4], F32, tag="shf")
            nc.vector.tensor_scalar(
                out=shf[:, :].bitcast(U32), in0=t4[:, :].bitcast(U32),
                scalar1=1, scalar2=None, op0=ALU.logical_shift_right)
            nc.vector.tensor_scalar(
                out=y.bitcast(U32), in0=shf[:, :].bitcast(U32), scalar1=-1.0,
                scalar2=float(RSQRT_MAGIC - 1), op0=ALU.mult, op1=ALU.add)
            tmp = st_p.tile([128, 4], F32, tag="nt_tmp")
            nc.vector.tensor_mul(tmp[:, :], y, y)
            nc.vector.tensor_mul(tmp[:, :], tmp[:, :], t4[:, :])
            nc.vector.tensor_scalar(
                out=tmp[:, :], in0=tmp[:, :], scalar1=-0.5, scalar2=1.5,
                op0=ALU.mult, op1=ALU.add)
            nc.vector.tensor_mul(y, y, tmp[:, :])

        def stats_to_pt(mv4, ptall):
            """From mv4 [128,4,2] (mean,var) build ptall [128,4,2] =
            (invs, -mu)."""
            nc.vector.tensor_scalar_mul(ptall[:, :, 1], mv4[:, :, 0], -1.0)
            t4 = st_p.tile([128, 4], F32, tag="t4")
            nc.vector.tensor_scalar_add(t4[:, :], mv4[:, :, 1], EPS)
            newton_rsqrt(t4, ptall)

        def load_z(it):
            """Allocate + DMA this tile's z data. Separate tiles per DMA so
            readers only wait on their own chunk's transfer."""
            bs = it * NT
            zt16 = zt_p.tile([K1_LAST + 2, NT], F16, tag="zt16")
            zr4 = []
            for bch in range(4):
                zc = zr_p.tile([128, D], F16, tag=f"zr_{bch}")
                nc.sync.dma_start(out=zc[:, :],
                                  in_=zr_d[bs + bch * 128:bs + (bch + 1) * 128, :])
                zr4.append(zc)
            ztg = []
            for g in range(4):
                zg = zt_p.tile([128, 4, NT], F16, tag=f"ztg_{g}")
                nc.sync.dma_start(
                    out=zg[:, :, :],
                    in_=zt_d[g * 512:(g + 1) * 512, bs:bs + NT].rearrange(
                        "(k p) n -> p k n", k=4))
                ztg.append(zg)
            nc.sync.dma_start(out=zt16[0:K1_LAST, :],
                              in_=zt_d[2048:2048 + K1_LAST, bs:bs + NT])
            return zr4, ztg, zt16

        pre_z = load_z(0)

        # ---- persistent constants / weights ----
        w1 = []
        for k in range(K1):
            rows = 128 if k < 16 else K1_LAST + 2
            t = wp.tile([rows, H], F16, tag=f"w1_{k}")
            nc.sync.dma_start(out=t[:, :], in_=w1_d[k * 128:k * 128 + rows, :])
            w1.append(t)
        w2 = []
        w3 = []
        aug2 = aug3 = None
        for name, wd, lst in (("w2", w2_d, w2), ("w3", w3_d, w3)):
            for k in range(4):
                t = wp.tile([128, H], F16, tag=f"{name}_{k}")
                nc.sync.dma_start(out=t[:, :], in_=wd[k * 128:(k + 1) * 128, :])
                lst.append(t)
            t = wp.tile([2, H], F16, tag=f"{name}_aug")
            nc.sync.dma_start(out=t[:, :], in_=wd[H:H + 2, :])
            if name == "w2":
                aug2 = t
            else:
                aug3 = t
        wo = wp.tile([128, 4], F16, tag="wo")
        for k in range(4):
            nc.sync.dma_start(out=wo[:, k:k + 1], in_=wo_d[k * 128:(k + 1) * 128, :])
        c1 = wp.tile([128, 4], F32, tag="c1")
        nc.sync.dma_start(out=c1[:, :], in_=c1_d[:, :])
        c2 = wp.tile([128, 4], F32, tag="c2")
        nc.sync.dma_start(out=c2[:, :], in_=c2_d[:, :])
        c3 = wp.tile([128, 4], F32, tag="c3")
        nc.sync.dma_start(out=c3[:, :], in_=c3_d[:, :])
        id32 = wp.tile([128, 128], F32, tag="id32")
        nc.sync.dma_start(out=id32[:, :], in_=i32_d[:, :])
        id16 = wp.tile([128, 128], F16, tag="id16")
        nc.sync.dma_start(out=id16[:, :], in_=i16_d[:, :])
        boutT = wp.tile([1, 1], F32, tag="boutT")
        nc.vector.memset(boutT[:, :], bout)
        qrow = wp.tile([1, ntiles * NT], F32, tag="qrow")


        for it in range(ntiles):
            bs = it * NT
            zr4, ztg, zt16 = pre_z if it == 0 else load_z(it)
            zts = [ztg[k // 4][:, k % 4, :] for k in range(16)]

            # ---- L1 stats: sum on ACT(Copy+accum), sumsq on ACT(Square+
            # accum) -> mean/var on DVE (keeps the in-order DVE free for
            # the latency-critical chain ops) ----
            s4 = st_p.tile([128, 4], F32, tag="s4")
            q4 = st_p.tile([128, 4], F32, tag="q4")
            for bch in range(4):
                scr = u_p.tile([128, D], F16, tag="scr")
                nc.scalar.activation(scr[:, :], zr4[bch][:, :], AF.Copy,
                                     accum_out=s4[:, bch:bch + 1])
                scr2 = u_p.tile([128, D], F16, tag="scr2")
                nc.scalar.activation(scr2[:, :], zr4[bch][:, :], AF.Square,
                                     accum_out=q4[:, bch:bch + 1])
            pt1 = st_p.tile([128, 4, 2], F32, tag="pt1")
            nc.vector.tensor_scalar_mul(pt1[:, :, 1], s4[:, :], -1.0 / D)
            m2 = st_p.tile([128, 4], F32, tag="m2")
            nc.vector.tensor_mul(m2[:, :], pt1[:, :, 1], pt1[:, :, 1])
            t4a = st_p.tile([128, 4], F32, tag="t4a")
            nc.vector.tensor_scalar(
                out=t4a[:, :], in0=q4[:, :], scalar1=1.0 / D, scalar2=EPS,
                op0=ALU.mult, op1=ALU.add)
            nc.vector.tensor_sub(t4a[:, :], t4a[:, :], m2[:, :])
            newton_rsqrt(t4a, pt1)
            i1row = st_p.tile([1, NT], F16, tag="i1row")
            for c in range(4):
                sl = slice(c * 128, (c + 1) * 128)
                ptrc = ps_tr.tile([2, 128], F32, tag="tr")
                nc.tensor.transpose(out=ptrc[:, :], in_=pt1[:, c, :],
                                    identity=id32[:, :])
                nc.vector.tensor_copy(out=zt16[K1_LAST:K1_LAST + 2, sl],
                                      in_=ptrc[0:2, :])
                nc.vector.tensor_copy(out=i1row[0:1, sl], in_=ptrc[0:1, :])
            bc1 = bc_p.tile([128, NT], F16, tag="bc1")
            nc.gpsimd.partition_broadcast(bc1[:, :], i1row[0:1, :])

            # ---- L1 matmuls + evac: all 64 mains first, then the four
            # k16 aug matmuls (which wait on the stats chain) — the chain
            # gets the whole main stretch as cover on the in-order PE ----
            h1 = []
            pys1 = []
            for m in range(4):
                py = ps_y.tile([128, NT], F32, tag="py")
                pys1.append(py)
                msl = slice(m * 128, (m + 1) * 128)
                for k in range(16):
                    nc.tensor.matmul(py[:, :], lhsT=w1[k][:, msl], rhs=zts[k],
                                     start=(k == 0), stop=False)
            for m in range(4):
                msl = slice(m * 128, (m + 1) * 128)
                nc.tensor.matmul(pys1[m][:, :], lhsT=w1[16][:, msl], rhs=zt16[:, :],
                                 start=False, stop=True)
            for m in range(4):
                u = u_p.tile([128, NT], F16, tag="u")
                nc.vector.tensor_mul(u[:, :], pys1[m][:, :], bc1[:, :])
                ht = h_p.tile([128, NT], F16, tag=f"h1_{m}")
                nc.scalar.activation(ht[:, :], u[:, :], AF.Tanh, bias=c1[:, m:m + 1])
                h1.append(ht)
            if dbg is not None and it == 0:
                nc.sync.dma_start(out=dbg["zt16"], in_=zt16[:, :])
                nc.sync.dma_start(out=dbg["bc1"], in_=bc1[:, :])
                for m in range(4):
                    nc.sync.dma_start(out=dbg["h1"][m * 128:(m + 1) * 128, :],
                                      in_=h1[m][:, :])

            # ---- L2 / L3 ----
            hcur = h1
            for lname, wts, augr, cv in (("l2", w2, aug2, c2), ("l3", w3, aug3, c3)):
                # batch-major stats: PE-transpose h into PSUM, bn_stats there
                st6 = st_p.tile([128, 4, 6], F32, tag=f"st6_{lname}")
                mv4b = st_p.tile([128, 4, 2], F32, tag=f"mv4_{lname}")
                for j in range(4):
                    jsl = slice(j * 128, (j + 1) * 128)
                    trj = ps_tr.tile([128, 512], F16, tag="tr")
                    for k in range(4):
                        nc.tensor.transpose(out=trj[:, k * 128:(k + 1) * 128],
                                            in_=hcur[k][:, jsl],
                                            identity=id16[:, :])
                    nc.vector.bn_stats(out=st6[:, j, :], in_=trj[:, :])
                    nc.vector.bn_aggr(out=mv4b[:, j, :], in_=st6[:, j, :])
                ptl = st_p.tile([128, 4, 2], F32, tag=f"pt_{lname}")
                stats_to_pt(mv4b, ptl)
                ivnm = h_p.tile([2, NT], F16, tag=f"ivnm_{lname}")
                for c in range(4):
                    sl = slice(c * 128, (c + 1) * 128)
                    ptrc = ps_tr.tile([2, 128], F32, tag="tr")
                    nc.tensor.transpose(out=ptrc[:, :], in_=ptl[:, c, :],
                                        identity=id32[:, :])
                    nc.vector.tensor_copy(out=ivnm[0:2, sl], in_=ptrc[0:2, :])
                bct = bc_p.tile([128, NT], F16, tag=f"bc_{lname}")
                nc.gpsimd.partition_broadcast(bct[:, :], ivnm[0:1, :])

                hnew = []
                pysl = []
                for m in range(4):
                    py = ps_y.tile([128, NT], F32, tag="py")
                    pysl.append(py)
                    msl = slice(m * 128, (m + 1) * 128)
                    for k in range(4):
                        nc.tensor.matmul(py[:, :], lhsT=wts[k][:, msl], rhs=hcur[k][:, :],
                                         start=(k == 0), stop=False)
                for m in range(4):
                    msl = slice(m * 128, (m + 1) * 128)
                    nc.tensor.matmul(pysl[m][:, :], lhsT=augr[:, msl], rhs=ivnm[:, :],
                                     start=False, stop=True)
                for m in range(4):
                    u = u_p.tile([128, NT], F16, tag="u")
                    nc.vector.tensor_mul(u[:, :], pysl[m][:, :], bct[:, :])
                    ht = h_p.tile([128, NT], F16, tag=f"h_{lname}_{m}")
                    nc.scalar.activation(ht[:, :], u[:, :], AF.Tanh, bias=cv[:, m:m + 1])
                    hnew.append(ht)
                if dbg is not None and it == 0 and lname == "l2":
                    nc.sync.dma_start(out=dbg["ivnm2"], in_=ivnm[:, :])
                    for m in range(4):
                        nc.sync.dma_start(out=dbg["h2"][m * 128:(m + 1) * 128, :],
                                          in_=hnew[m][:, :])
                hcur = hnew

            # ---- L4 ----
            pq = ps_q.tile([1, NT], F32, tag="pq")
            for k in range(4):
                nc.tensor.matmul(pq[:, :], lhsT=wo[:, k:k + 1], rhs=hcur[k][:, :],
                                 start=(k == 0), stop=(k == 3))
            nc.scalar.activation(qrow[0:1, bs:bs + NT], pq[:, :], AF.Tanh, bias=boutT[:, :])

        nc.sync.dma_start(out=q_d[:, :], in_=qrow[:, :])


# ---------------- host side ----------------

def host_prep(x, a, g1, beta1, g2, beta2, g3, beta3,
              w1, b1, w2, b2, w3, b3, w_out, b_out):
    """Shared (replicated) tensors + full z arrays; returns dict pieces."""
    f16 = np.float16
    z = np.empty((x.shape[0], D), dtype=f16)
    np.multiply(x[:, :HALF], np.float32(1.0 / X_NORM), out=z[:, :HALF], casting="unsafe")
    np.multiply(x[:, HALF:], np.float32(1.0 / V_NORM), out=z[:, HALF:INPUT_DIM], casting="unsafe")
    z[:, INPUT_DIM:] = a.astype(f16)

    def fold(w, g, beta, b):
        wg = (w.astype(np.float64) * g.astype(np.float64)[None, :])
        rs = wg.sum(axis=1)
        c = w.astype(np.float64) @ beta.astype(np.float64) + b.astype(np.float64)
        out = np.empty((w.shape[1] + 2, w.shape[0]), dtype=f16)
        out[:w.shape[1]] = wg.T.astype(f16)
        # device aug rows arrive as (invs, -mu) -> weight rows (0, rs)
        out[w.shape[1]] = 0.0
        out[w.shape[1] + 1] = rs.astype(f16)
        cdev = np.ascontiguousarray(
            c.astype(np.float32).reshape(4, 128).T)  # [128, 4]
        return out, cdev

    w1a, c1 = fold(w1, g1, beta1, b1)
    w2a, c2 = fold(w2, g2, beta2, b2)
    w3a, c3 = fold(w3, g3, beta3, b3)
    wout = w_out.T.astype(f16)  # [H, 1]
    bout = float(b_out[0])
    ident32 = np.eye(128, dtype=np.float32)
    ident16 = np.eye(128, dtype=f16)
    return z, w1a, w2a, w3a, wout, bout, c1, c2, c3, ident32, ident16


def build_in_maps(inputs):
    z, w1a, w2a, w3a, wout, bout, c1, c2, c3, id32, id16 = host_prep(**inputs)
    in_maps = []
    for c in range(NCORES):
        zc = z[c * BC:(c + 1) * BC]
        in_maps.append({
            "zr": np.ascontiguousarray(zc),
            "zt": np.ascontiguousarray(zc.T),
            "w1a": w1a, "w2a": w2a, "w3a": w3a, "wout": wout,
            "c1": c1, "c2": c2, "c3": c3,
            "ident32": id32, "ident16": id16,
        })
    return in_maps, bout


_NC_CACHE = {}


def get_nc(bout: float):
    key = (round(bout, 10), BC)
    if key not in _NC_CACHE:
        _NC_CACHE[key] = build_nc(bout, BC)
    return _NC_CACHE[key]


def kernel(**inputs):
    inputs = {k: np.asarray(v) for k, v in inputs.items()}
    in_maps, bout = build_in_maps(inputs)
    nc = get_nc(bout)
    res = run_bass_kernel_spmd(nc, in_maps, list(range(NCORES)))
    q = np.concatenate([res.results[c]["q"].reshape(BC, 1) for c in range(NCORES)],
                       axis=0).astype(np.float32)
    return q
